# revision 28
# baseline (speedup 1.0000x reference)
"""DCRNN (K=1, H0=0) fused kernel for 8 Trainium2 NeuronCores.

Math (derived from the reference with H0 = 0):
    R is dead (multiplied by H0=0); XH == XHR == [x, 0].
    Az = (Wz[0] + Wz[1])[:F]           # [256, 32]
    Ah = (Wh[0] + Wh[1])[:F]           # [256, 32]
    Zc = sigmoid(-(x @ Az + bz))       # == 1 - Z, strictly positive
    T  = tanh(x @ Ah + bh) = 2*sigmoid(2(x@Ah+bh)) - 1
    h  = relu(Zc * T) == Zc * relu(T)
    y  = h @ Wl + bl                   # [N, 1]

Key tricks vs the old baseline:
  * Mixed precision x: features 0-127 in bf16, features 128-255 in fp8
    e3m4 (scaled by 2 to dodge the subnormal zone; weights pre-divided
    by 2).  HBM traffic drops to 75% (384 B/node).  Measured rel err
    ~1.3e-2 vs the 2e-2 gate.
  * Single activation function: fold -1 into the Az columns and +2 into
    the Ah columns so both gates are plain sigmoid() of the psum -- no
    sigmoid<->tanh ACT table reloads.  tanh path becomes
    (sh - 0.5) * zc * (2*Wl), with the 2 folded into Wl host-side.
  * Biases are zero in this model; the rank-1 bias matmuls are only
    emitted when any bias is nonzero.
  * DMA in 2048-node megablocks (512 KB + 256 KB per block) on the two
    HWDGE queues (sync + scalar) for near-peak HBM bandwidth.
"""

import sys

import numpy as np

sys.path.insert(0, "/opt/trn_rl_repo")

import ml_dtypes

N = 200000
F = 256
HID = 32
NCORES = 8
PER = 25088            # padded nodes per core
NPAD = PER * NCORES    # 200704
MEGA = 4096            # max nodes per megablock (32 chunks of 128)
BLOCKS = [32, 32, 32, 32, 32, 32, 4]
assert sum(BLOCKS) == 196
YCOLS = PER // 128     # 196

BF16 = ml_dtypes.bfloat16
F8E3 = ml_dtypes.float8_e3m4

_PROGS = {}


def _build_program(has_bias=False):
    import concourse.tile as tile
    from concourse import bacc, mybir

    BF = mybir.dt.bfloat16
    F8 = mybir.dt.float8e3
    F32 = mybir.dt.float32
    AF = mybir.ActivationFunctionType
    OP = mybir.AluOpType

    nc = bacc.Bacc("TRN2", target_bir_lowering=False, debug=False,
                   num_devices=NCORES)

    # host feeds per-megablock transposed contiguous blocks
    x0_d = nc.dram_tensor("x0", [PER * 128], BF, kind="ExternalInput").ap()
    x1_d = nc.dram_tensor("x1", [PER * 128], F8, kind="ExternalInput").ap()
    acat_d = nc.dram_tensor("acat", [2, 128, 64], BF, kind="ExternalInput").ap()
    bias_d = nc.dram_tensor("biascat", [1, 512], BF, kind="ExternalInput").ap()
    wl_d = nc.dram_tensor("wlfull", [128, 1024], BF, kind="ExternalInput").ap()
    ones_d = nc.dram_tensor("ones", [1, 128], BF, kind="ExternalInput").ap()
    id_d = nc.dram_tensor("ident", [128, 128], BF, kind="ExternalInput").ap()
    y_d = nc.dram_tensor("y", [YCOLS, 128], F32, kind="ExternalOutput").ap()

    with tile.TileContext(nc) as tc:
        with tc.tile_pool(name="const", bufs=1) as cp, \
             tc.tile_pool(name="x0t", bufs=4) as xp0, \
             tc.tile_pool(name="x1t", bufs=len(BLOCKS)) as xp1, \
             tc.tile_pool(name="zs", bufs=3) as zp, \
             tc.tile_pool(name="act", bufs=4) as vp, \
             tc.tile_pool(name="ps", bufs=6, space="PSUM") as pp, \
             tc.tile_pool(name="yps", bufs=2, space="PSUM") as yp:

            acat0 = cp.tile([128, 64], BF)
            acat1 = cp.tile([128, 64], BF)
            biascat = cp.tile([1, 512], BF)
            wlfull = cp.tile([128, 1024], BF)
            ones = cp.tile([1, 128], BF)
            ident = cp.tile([128, 128], BF)
            ysb = cp.tile([128, YCOLS], BF)

            nc.scalar.dma_start(out=acat0[:], in_=acat_d[0])
            nc.scalar.dma_start(out=acat1[:], in_=acat_d[1])
            nc.scalar.dma_start(out=biascat[:], in_=bias_d[:])
            nc.scalar.dma_start(out=wlfull[:], in_=wl_d[:])
            nc.scalar.dma_start(out=ones[:], in_=ones_d[:])
            nc.scalar.dma_start(out=ident[:], in_=id_d[:])

            # pre-issue ALL fp8 x1 loads on the scalar HWDGE ring (dedicated
            # buffers, no WAR waits) so the two HWDGE rings stream in parallel
            xt1s = []
            pos = 0
            for nchunk in BLOCKS:
                nn = nchunk * 128
                off = pos * 128 * 128
                t = xp1.tile([128, MEGA], F8, tag="xt1")
                nc.scalar.dma_start(
                    out=t[:, :nn],
                    in_=x1_d[off:off + 128 * nn].rearrange(
                        "(p j) -> p j", p=128))
                xt1s.append(t)
                pos += nchunk

            ycol = 0
            for b, nchunk in enumerate(BLOCKS):
                nn = nchunk * 128
                off = ycol * 128 * 128

                xt0 = xp0.tile([128, MEGA], BF, tag="xt0")
                xt1 = xt1s[b]
                nc.sync.dma_start(
                    out=xt0[:, :nn],
                    in_=x0_d[off:off + 128 * nn].rearrange(
                        "(p j) -> p j", p=128))

                # zs layout: [zc for all chunks | sh for all chunks]
                zs = zp.tile([128, MEGA // 2], BF, tag="zs")
                zs2 = zs.rearrange("p (t q) -> p t q", t=2)
                for h in range(0, nchunk, 8):
                    hs = min(8, nchunk - h)
                    ps = pp.tile([128, 512], F32, tag="ps")
                    # de-interleaved psum: cols 0-255 zc-pre, 256-511 sh-pre
                    ps2 = ps.rearrange("p (t q) -> p t q", t=2)
                    if has_bias:
                        nc.tensor.matmul(ps2[:, :, :hs * 32], ones[:],
                                         biascat.rearrange(
                                             "p (t q) -> p t q",
                                             t=2)[:, :, :hs * 32],
                                         start=True, stop=False)
                    for s in range(hs):
                        c = h + s
                        out_sl = ps2[:, :, s * 32:(s + 1) * 32]
                        nc.tensor.matmul(
                            out_sl, xt0[:, c * 128:(c + 1) * 128], acat0[:],
                            start=not has_bias, stop=False)
                        nc.tensor.matmul(
                            out_sl, xt1[:, c * 128:(c + 1) * 128], acat1[:],
                            start=False, stop=True)

                    nc.scalar.activation(
                        zs2[:, :, h * 32:h * 32 + hs * 32],
                        ps2[:, :, :hs * 32], AF.Sigmoid)

                # batched gating over the whole megablock (dense bf16 ops;
                # relu/mul/reduce run in DVE 4x/2x perf modes)
                t1 = vp.tile([128, MEGA // 4], BF, tag="t1")
                # t1 = (sh - 0.5) * zc ; relu deferred (zc > 0)
                nc.vector.scalar_tensor_tensor(
                    t1[:, :nchunk * 32],
                    zs[:, MEGA // 4:MEGA // 4 + nchunk * 32], 0.5,
                    zs[:, :nchunk * 32],
                    op0=OP.subtract, op1=OP.mult)
                rl = vp.tile([128, MEGA // 4], BF, tag="rl")
                nc.vector.tensor_scalar_max(rl[:, :nchunk * 32],
                                            t1[:, :nchunk * 32], 0.0)
                gw = vp.tile([128, MEGA // 4], BF, tag="gw")
                nc.vector.tensor_mul(gw[:, :nchunk * 32], rl[:, :nchunk * 32],
                                     wlfull[:, :nchunk * 32])
                gw3 = gw[:, :nchunk * 32].rearrange("p (s g) -> p s g", g=32)
                with nc.allow_low_precision(
                        reason="DVE reduce accumulates fp32 internally; "
                               "bf16 output validated vs reference"):
                    nc.vector.tensor_reduce(ysb[:, ycol:ycol + nchunk], gw3,
                                            axis=mybir.AxisListType.X,
                                            op=OP.add)

                # flush this megablock's ysb columns to keep the tail short
                ytp = yp.tile([128, 128], BF, tag="ytp")
                nc.tensor.transpose(ytp[:nchunk, :],
                                    ysb[:, ycol:ycol + nchunk], ident[:])
                yts = vp.tile([128, 128], F32, tag="yts")
                nc.vector.tensor_copy(yts[:nchunk, :], ytp[:nchunk, :])
                nc.gpsimd.dma_start(out=y_d[ycol:ycol + nchunk, :],
                                    in_=yts[:nchunk, :])
                ycol += nchunk

    nc.compile()
    return nc


def _get_program(has_bias=False):
    if has_bias not in _PROGS:
        _PROGS[has_bias] = _build_program(has_bias)
    return _PROGS[has_bias]


def _host_inputs(x, Wz, bz, Wh, bh, Wl):
    Az = (np.asarray(Wz[0], np.float32) + np.asarray(Wz[1], np.float32))[:F]
    Ah = (np.asarray(Wh[0], np.float32) + np.asarray(Wh[1], np.float32))[:F]
    Acat = np.concatenate([Az, Ah], axis=1)               # [256, 64]
    colscale = np.concatenate([-np.ones(32, np.float32),
                               2 * np.ones(32, np.float32)])
    Acat = Acat * colscale
    acat = np.stack([Acat[:128], Acat[128:] * 0.5]).astype(BF16)
    bsc = np.concatenate([np.asarray(bz, np.float32),
                          np.asarray(bh, np.float32)]) \
        .reshape(2, HID) * colscale.reshape(2, HID)
    # de-interleaved: [bz-scaled x8 | bh-scaled x8]
    biascat8 = np.concatenate([np.tile(bsc[0], 8), np.tile(bsc[1], 8)]) \
        [None, :].astype(BF16)
    wlfull = np.tile(2.0 * np.asarray(Wl, np.float32).reshape(1, HID),
                     (128, 32)).astype(BF16)
    ones = np.ones((1, 128), BF16)
    ident = np.eye(128, dtype=BF16)

    xf = np.asarray(x, np.float32)
    xb0 = np.zeros((NPAD, 128), dtype=BF16)
    xb0[:N] = xf[:, :128].astype(BF16)
    xb1 = np.zeros((NPAD, 128), dtype=F8E3)
    xb1[:N] = (2.0 * xf[:, 128:]).astype(F8E3)

    # per-core shards, then per-megablock transposed contiguous blocks
    def mega_layout(xb):
        sh = xb.reshape(NCORES, PER, 128)
        parts = []
        pos = 0
        for nchunk in BLOCKS:
            nn = nchunk * 128
            blk = sh[:, pos:pos + nn]  # [NCORES, nn, 128]
            parts.append(np.ascontiguousarray(
                blk.transpose(0, 2, 1)).reshape(NCORES, -1))
            pos += nn
        return np.concatenate(parts, axis=1)  # [NCORES, PER*128]

    return (mega_layout(xb0), mega_layout(xb1), acat, biascat8, wlfull,
            ones, ident)


def kernel(x, edge_index, Wz, bz, Wr, br, Wh, bh, Wl, bl):
    from concourse.bass_utils import run_bass_kernel_spmd

    s0, s1, acat, biascat8, wlfull, ones, ident = _host_inputs(
        x, Wz, bz, Wh, bh, Wl)
    has_bias = bool(np.any(np.asarray(bz)) or np.any(np.asarray(bh)))

    nc = _get_program(has_bias)
    in_maps = [{
        "x0": np.ascontiguousarray(s0[i]),
        "x1": np.ascontiguousarray(s1[i]),
        "acat": acat,
        "biascat": biascat8,
        "wlfull": wlfull,
        "ones": ones,
        "ident": ident,
    } for i in range(NCORES)]

    res = run_bass_kernel_spmd(nc, in_maps, core_ids=list(range(NCORES)))

    y = np.concatenate([np.asarray(res.results[i]["y"]).reshape(-1)
                        for i in range(NCORES)])[:N]
    out = (y + np.float32(np.asarray(bl).reshape(-1)[0])).astype(np.float32)
    return out.reshape(N, 1)


# revision 29
# speedup vs baseline: 1.0321x; 1.0321x over previous
"""DCRNN (K=1, H0=0) fused kernel for 8 Trainium2 NeuronCores.

Math (derived from the reference with H0 = 0):
    R is dead (multiplied by H0=0); XH == XHR == [x, 0].
    Az = (Wz[0] + Wz[1])[:F]           # [256, 32]
    Ah = (Wh[0] + Wh[1])[:F]           # [256, 32]
    Zc = sigmoid(-(x @ Az + bz))       # == 1 - Z, strictly positive
    T  = tanh(x @ Ah + bh) = 2*sigmoid(2(x@Ah+bh)) - 1
    h  = relu(Zc * T) == Zc * relu(T)
    y  = h @ Wl + bl                   # [N, 1]

Key tricks vs the old baseline:
  * Mixed precision x: features 0-127 in bf16, features 128-255 in fp8
    e3m4 (scaled by 2 to dodge the subnormal zone; weights pre-divided
    by 2).  HBM traffic drops to 75% (384 B/node).  Measured rel err
    ~1.3e-2 vs the 2e-2 gate.
  * Single activation function: fold -1 into the Az columns and +2 into
    the Ah columns so both gates are plain sigmoid() of the psum -- no
    sigmoid<->tanh ACT table reloads.  tanh path becomes
    (sh - 0.5) * zc * (2*Wl), with the 2 folded into Wl host-side.
  * Biases are zero in this model; the rank-1 bias matmuls are only
    emitted when any bias is nonzero.
  * DMA in 2048-node megablocks (512 KB + 256 KB per block) on the two
    HWDGE queues (sync + scalar) for near-peak HBM bandwidth.
"""

import sys

import numpy as np

sys.path.insert(0, "/opt/trn_rl_repo")

import ml_dtypes

N = 200000
F = 256
HID = 32
NCORES = 8
PER = 25088            # padded nodes per core
NPAD = PER * NCORES    # 200704
MEGA = 8192            # max nodes per megablock (64 chunks of 128)
BLOCKS = [64, 64, 48, 16, 4]
assert sum(BLOCKS) == 196
YCOLS = PER // 128     # 196

BF16 = ml_dtypes.bfloat16
F8E3 = ml_dtypes.float8_e3m4

_PROGS = {}


def _build_program(has_bias=False):
    import concourse.tile as tile
    from concourse import bacc, mybir

    BF = mybir.dt.bfloat16
    F8 = mybir.dt.float8e3
    F32 = mybir.dt.float32
    AF = mybir.ActivationFunctionType
    OP = mybir.AluOpType

    nc = bacc.Bacc("TRN2", target_bir_lowering=False, debug=False,
                   num_devices=NCORES)

    # host feeds per-megablock transposed contiguous blocks
    x0_d = nc.dram_tensor("x0", [PER * 128], BF, kind="ExternalInput").ap()
    x1_d = nc.dram_tensor("x1", [PER * 128], F8, kind="ExternalInput").ap()
    acat_d = nc.dram_tensor("acat", [2, 128, 64], BF, kind="ExternalInput").ap()
    bias_d = nc.dram_tensor("biascat", [1, 512], BF, kind="ExternalInput").ap()
    wl_d = nc.dram_tensor("wlfull", [128, 2048], BF, kind="ExternalInput").ap()
    ones_d = nc.dram_tensor("ones", [1, 128], BF, kind="ExternalInput").ap()
    id_d = nc.dram_tensor("ident", [128, 128], BF, kind="ExternalInput").ap()
    y_d = nc.dram_tensor("y", [YCOLS, 128], F32, kind="ExternalOutput").ap()

    with tile.TileContext(nc) as tc:
        with tc.tile_pool(name="const", bufs=1) as cp, \
             tc.tile_pool(name="x0t", bufs=3) as xp0, \
             tc.tile_pool(name="x1t", bufs=3) as xp1, \
             tc.tile_pool(name="zs", bufs=2) as zp, \
             tc.tile_pool(name="act", bufs=3) as vp, \
             tc.tile_pool(name="ps", bufs=6, space="PSUM") as pp, \
             tc.tile_pool(name="yps", bufs=2, space="PSUM") as yp:

            acat0 = cp.tile([128, 64], BF)
            acat1 = cp.tile([128, 64], BF)
            biascat = cp.tile([1, 512], BF)
            wlfull = cp.tile([128, 2048], BF)
            ones = cp.tile([1, 128], BF)
            ident = cp.tile([128, 128], BF)
            ysb = cp.tile([128, YCOLS], BF)

            nc.scalar.dma_start(out=acat0[:], in_=acat_d[0])
            nc.scalar.dma_start(out=acat1[:], in_=acat_d[1])
            nc.scalar.dma_start(out=biascat[:], in_=bias_d[:])
            nc.scalar.dma_start(out=wlfull[:], in_=wl_d[:])
            nc.scalar.dma_start(out=ones[:], in_=ones_d[:])
            nc.scalar.dma_start(out=ident[:], in_=id_d[:])

            ycol = 0
            for b, nchunk in enumerate(BLOCKS):
                nn = nchunk * 128
                off = ycol * 128 * 128

                xt0 = xp0.tile([128, MEGA], BF, tag="xt0")
                xt1 = xp1.tile([128, MEGA], F8, tag="xt1")
                nc.sync.dma_start(
                    out=xt0[:, :nn],
                    in_=x0_d[off:off + 128 * nn].rearrange(
                        "(p j) -> p j", p=128))
                nc.sync.dma_start(
                    out=xt1[:, :nn],
                    in_=x1_d[off:off + 128 * nn].rearrange(
                        "(p j) -> p j", p=128))

                # zs layout: [zc for all chunks | sh for all chunks]
                zs = zp.tile([128, MEGA // 2], BF, tag="zs")
                zs2 = zs.rearrange("p (t q) -> p t q", t=2)
                for h in range(0, nchunk, 8):
                    hs = min(8, nchunk - h)
                    ps = pp.tile([128, 512], F32, tag="ps")
                    # de-interleaved psum: cols 0-255 zc-pre, 256-511 sh-pre
                    ps2 = ps.rearrange("p (t q) -> p t q", t=2)
                    if has_bias:
                        nc.tensor.matmul(ps2[:, :, :hs * 32], ones[:],
                                         biascat.rearrange(
                                             "p (t q) -> p t q",
                                             t=2)[:, :, :hs * 32],
                                         start=True, stop=False)
                    for s in range(hs):
                        c = h + s
                        out_sl = ps2[:, :, s * 32:(s + 1) * 32]
                        nc.tensor.matmul(
                            out_sl, xt0[:, c * 128:(c + 1) * 128], acat0[:],
                            start=not has_bias, stop=False)
                        nc.tensor.matmul(
                            out_sl, xt1[:, c * 128:(c + 1) * 128], acat1[:],
                            start=False, stop=True)

                    nc.scalar.activation(
                        zs2[:, :, h * 32:h * 32 + hs * 32],
                        ps2[:, :, :hs * 32], AF.Sigmoid)

                # batched gating over the whole megablock (dense bf16 ops;
                # relu/mul/reduce run in DVE 4x/2x perf modes)
                t1 = vp.tile([128, MEGA // 4], BF, tag="t1")
                # t1 = (sh - 0.5) * zc ; relu deferred (zc > 0)
                nc.vector.scalar_tensor_tensor(
                    t1[:, :nchunk * 32],
                    zs[:, MEGA // 4:MEGA // 4 + nchunk * 32], 0.5,
                    zs[:, :nchunk * 32],
                    op0=OP.subtract, op1=OP.mult)
                rl = vp.tile([128, MEGA // 4], BF, tag="rl")
                nc.vector.tensor_scalar_max(rl[:, :nchunk * 32],
                                            t1[:, :nchunk * 32], 0.0)
                gw = vp.tile([128, MEGA // 4], BF, tag="gw")
                nc.vector.tensor_mul(gw[:, :nchunk * 32], rl[:, :nchunk * 32],
                                     wlfull[:, :nchunk * 32])
                gw3 = gw[:, :nchunk * 32].rearrange("p (s g) -> p s g", g=32)
                with nc.allow_low_precision(
                        reason="DVE reduce accumulates fp32 internally; "
                               "bf16 output validated vs reference"):
                    nc.vector.tensor_reduce(ysb[:, ycol:ycol + nchunk], gw3,
                                            axis=mybir.AxisListType.X,
                                            op=OP.add)

                # flush this megablock's ysb columns to keep the tail short
                ytp = yp.tile([128, 128], BF, tag="ytp")
                nc.tensor.transpose(ytp[:nchunk, :],
                                    ysb[:, ycol:ycol + nchunk], ident[:])
                yts = vp.tile([128, 128], F32, tag="yts")
                nc.vector.tensor_copy(yts[:nchunk, :], ytp[:nchunk, :])
                nc.gpsimd.dma_start(out=y_d[ycol:ycol + nchunk, :],
                                    in_=yts[:nchunk, :])
                ycol += nchunk

    nc.compile()
    return nc


def _get_program(has_bias=False):
    if has_bias not in _PROGS:
        _PROGS[has_bias] = _build_program(has_bias)
    return _PROGS[has_bias]


def _host_inputs(x, Wz, bz, Wh, bh, Wl):
    Az = (np.asarray(Wz[0], np.float32) + np.asarray(Wz[1], np.float32))[:F]
    Ah = (np.asarray(Wh[0], np.float32) + np.asarray(Wh[1], np.float32))[:F]
    Acat = np.concatenate([Az, Ah], axis=1)               # [256, 64]
    colscale = np.concatenate([-np.ones(32, np.float32),
                               2 * np.ones(32, np.float32)])
    Acat = Acat * colscale
    acat = np.stack([Acat[:128], Acat[128:] * 0.5]).astype(BF16)
    bsc = np.concatenate([np.asarray(bz, np.float32),
                          np.asarray(bh, np.float32)]) \
        .reshape(2, HID) * colscale.reshape(2, HID)
    # de-interleaved: [bz-scaled x8 | bh-scaled x8]
    biascat8 = np.concatenate([np.tile(bsc[0], 8), np.tile(bsc[1], 8)]) \
        [None, :].astype(BF16)
    wlfull = np.tile(2.0 * np.asarray(Wl, np.float32).reshape(1, HID),
                     (128, 64)).astype(BF16)
    ones = np.ones((1, 128), BF16)
    ident = np.eye(128, dtype=BF16)

    xf = np.asarray(x, np.float32)
    xb0 = np.zeros((NPAD, 128), dtype=BF16)
    xb0[:N] = xf[:, :128].astype(BF16)
    xb1 = np.zeros((NPAD, 128), dtype=F8E3)
    xb1[:N] = (2.0 * xf[:, 128:]).astype(F8E3)

    # per-core shards, then per-megablock transposed contiguous blocks
    def mega_layout(xb):
        sh = xb.reshape(NCORES, PER, 128)
        parts = []
        pos = 0
        for nchunk in BLOCKS:
            nn = nchunk * 128
            blk = sh[:, pos:pos + nn]  # [NCORES, nn, 128]
            parts.append(np.ascontiguousarray(
                blk.transpose(0, 2, 1)).reshape(NCORES, -1))
            pos += nn
        return np.concatenate(parts, axis=1)  # [NCORES, PER*128]

    return (mega_layout(xb0), mega_layout(xb1), acat, biascat8, wlfull,
            ones, ident)


def kernel(x, edge_index, Wz, bz, Wr, br, Wh, bh, Wl, bl):
    from concourse.bass_utils import run_bass_kernel_spmd

    s0, s1, acat, biascat8, wlfull, ones, ident = _host_inputs(
        x, Wz, bz, Wh, bh, Wl)
    has_bias = bool(np.any(np.asarray(bz)) or np.any(np.asarray(bh)))

    nc = _get_program(has_bias)
    in_maps = [{
        "x0": np.ascontiguousarray(s0[i]),
        "x1": np.ascontiguousarray(s1[i]),
        "acat": acat,
        "biascat": biascat8,
        "wlfull": wlfull,
        "ones": ones,
        "ident": ident,
    } for i in range(NCORES)]

    res = run_bass_kernel_spmd(nc, in_maps, core_ids=list(range(NCORES)))

    y = np.concatenate([np.asarray(res.results[i]["y"]).reshape(-1)
                        for i in range(NCORES)])[:N]
    out = (y + np.float32(np.asarray(bl).reshape(-1)[0])).astype(np.float32)
    return out.reshape(N, 1)


# revision 30
# speedup vs baseline: 1.2058x; 1.1683x over previous
"""DCRNN (K=1, H0=0) fused kernel for 8 Trainium2 NeuronCores.

Math (derived from the reference with H0 = 0):
    R is dead (multiplied by H0=0); XH == XHR == [x, 0].
    Az = (Wz[0] + Wz[1])[:F]           # [256, 32]
    Ah = (Wh[0] + Wh[1])[:F]           # [256, 32]
    Zc = sigmoid(-(x @ Az + bz))       # == 1 - Z, strictly positive
    T  = tanh(x @ Ah + bh) = 2*sigmoid(2(x@Ah+bh)) - 1
    h  = relu(Zc * T) == Zc * relu(T)
    y  = h @ Wl + bl                   # [N, 1]

Key tricks vs the old baseline:
  * Mixed precision x: features 0-127 in bf16, features 128-255 in fp8
    e3m4 (scaled by 2 to dodge the subnormal zone; weights pre-divided
    by 2).  HBM traffic drops to 75% (384 B/node).  Measured rel err
    ~1.3e-2 vs the 2e-2 gate.
  * Single activation function: fold -1 into the Az columns and +2 into
    the Ah columns so both gates are plain sigmoid() of the psum -- no
    sigmoid<->tanh ACT table reloads.  tanh path becomes
    (sh - 0.5) * zc * (2*Wl), with the 2 folded into Wl host-side.
  * Biases are zero in this model; the rank-1 bias matmuls are only
    emitted when any bias is nonzero.
  * DMA in 2048-node megablocks (512 KB + 256 KB per block) on the two
    HWDGE queues (sync + scalar) for near-peak HBM bandwidth.
"""

import sys

import numpy as np

sys.path.insert(0, "/opt/trn_rl_repo")

import ml_dtypes

N = 200000
F = 256
HID = 32
NCORES = 8
PER = 25088            # padded nodes per core
NPAD = PER * NCORES    # 200704
MEGA = 4096            # max nodes per megablock (32 chunks of 128)
BLOCKS = [32, 32, 32, 32, 32, 32, 4]
assert sum(BLOCKS) == 196
YCOLS = PER // 128     # 196

BF16 = ml_dtypes.bfloat16
F8E3 = ml_dtypes.float8_e3m4

_PROGS = {}


def _build_program(has_bias=False):
    import concourse.tile as tile
    from concourse import bacc, mybir

    BF = mybir.dt.bfloat16
    F8 = mybir.dt.float8e3
    F32 = mybir.dt.float32
    AF = mybir.ActivationFunctionType
    OP = mybir.AluOpType

    nc = bacc.Bacc("TRN2", target_bir_lowering=False, debug=False,
                   num_devices=NCORES)

    # host feeds per-megablock transposed contiguous blocks
    x0_d = nc.dram_tensor("x0", [PER * 128], F8, kind="ExternalInput").ap()
    x1_d = nc.dram_tensor("x1", [PER * 128], F8, kind="ExternalInput").ap()
    acat_d = nc.dram_tensor("acat", [2, 128, 64], BF, kind="ExternalInput").ap()
    bias_d = nc.dram_tensor("biascat", [1, 512], BF, kind="ExternalInput").ap()
    wl_d = nc.dram_tensor("wlfull", [128, 2048], BF, kind="ExternalInput").ap()
    ones_d = nc.dram_tensor("ones", [1, 128], BF, kind="ExternalInput").ap()
    id_d = nc.dram_tensor("ident", [128, 128], BF, kind="ExternalInput").ap()
    y_d = nc.dram_tensor("y", [YCOLS, 128], F32, kind="ExternalOutput").ap()

    with tile.TileContext(nc) as tc:
        with tc.tile_pool(name="const", bufs=1) as cp, \
             tc.tile_pool(name="x0t", bufs=3) as xp0, \
             tc.tile_pool(name="x1t", bufs=3) as xp1, \
             tc.tile_pool(name="zs", bufs=2) as zp, \
             tc.tile_pool(name="act", bufs=3) as vp, \
             tc.tile_pool(name="ps", bufs=6, space="PSUM") as pp, \
             tc.tile_pool(name="yps", bufs=2, space="PSUM") as yp:

            acat0 = cp.tile([128, 64], BF)
            acat1 = cp.tile([128, 64], BF)
            biascat = cp.tile([1, 512], BF)
            wlfull = cp.tile([128, 2048], BF)
            ones = cp.tile([1, 128], BF)
            ident = cp.tile([128, 128], BF)
            ysb = cp.tile([128, YCOLS], BF)

            nc.scalar.dma_start(out=acat0[:], in_=acat_d[0])
            nc.scalar.dma_start(out=acat1[:], in_=acat_d[1])
            nc.scalar.dma_start(out=biascat[:], in_=bias_d[:])
            nc.scalar.dma_start(out=wlfull[:], in_=wl_d[:])
            nc.scalar.dma_start(out=ones[:], in_=ones_d[:])
            nc.scalar.dma_start(out=ident[:], in_=id_d[:])

            ycol = 0
            for b, nchunk in enumerate(BLOCKS):
                nn = nchunk * 128
                off = ycol * 128 * 128

                xt0 = xp0.tile([128, MEGA], F8, tag="xt0")
                xt1 = xp1.tile([128, MEGA], F8, tag="xt1")
                nc.sync.dma_start(
                    out=xt0[:, :nn],
                    in_=x0_d[off:off + 128 * nn].rearrange(
                        "(p j) -> p j", p=128))
                nc.sync.dma_start(
                    out=xt1[:, :nn],
                    in_=x1_d[off:off + 128 * nn].rearrange(
                        "(p j) -> p j", p=128))

                # zs layout: [zc for all chunks | sh for all chunks]
                zs = zp.tile([128, MEGA // 2], BF, tag="zs")
                zs2 = zs.rearrange("p (t q) -> p t q", t=2)
                for h in range(0, nchunk, 8):
                    hs = min(8, nchunk - h)
                    ps = pp.tile([128, 512], F32, tag="ps")
                    # de-interleaved psum: cols 0-255 zc-pre, 256-511 sh-pre
                    ps2 = ps.rearrange("p (t q) -> p t q", t=2)
                    if has_bias:
                        nc.tensor.matmul(ps2[:, :, :hs * 32], ones[:],
                                         biascat.rearrange(
                                             "p (t q) -> p t q",
                                             t=2)[:, :, :hs * 32],
                                         start=True, stop=False)
                    for s in range(hs):
                        c = h + s
                        out_sl = ps2[:, :, s * 32:(s + 1) * 32]
                        nc.tensor.matmul(
                            out_sl, xt0[:, c * 128:(c + 1) * 128], acat0[:],
                            start=not has_bias, stop=False)
                        nc.tensor.matmul(
                            out_sl, xt1[:, c * 128:(c + 1) * 128], acat1[:],
                            start=False, stop=True)

                    nc.scalar.activation(
                        zs2[:, :, h * 32:h * 32 + hs * 32],
                        ps2[:, :, :hs * 32], AF.Sigmoid)

                # batched gating over the whole megablock (dense bf16 ops;
                # relu/mul/reduce run in DVE 4x/2x perf modes)
                t1 = vp.tile([128, MEGA // 4], BF, tag="t1")
                # t1 = (sh - 0.5) * zc ; relu deferred (zc > 0)
                nc.vector.scalar_tensor_tensor(
                    t1[:, :nchunk * 32],
                    zs[:, MEGA // 4:MEGA // 4 + nchunk * 32], 0.5,
                    zs[:, :nchunk * 32],
                    op0=OP.subtract, op1=OP.mult)
                rl = vp.tile([128, MEGA // 4], BF, tag="rl")
                nc.vector.tensor_scalar_max(rl[:, :nchunk * 32],
                                            t1[:, :nchunk * 32], 0.0)
                gw = vp.tile([128, MEGA // 4], BF, tag="gw")
                nc.vector.tensor_mul(gw[:, :nchunk * 32], rl[:, :nchunk * 32],
                                     wlfull[:, :nchunk * 32])
                gw3 = gw[:, :nchunk * 32].rearrange("p (s g) -> p s g", g=32)
                with nc.allow_low_precision(
                        reason="DVE reduce accumulates fp32 internally; "
                               "bf16 output validated vs reference"):
                    nc.vector.tensor_reduce(ysb[:, ycol:ycol + nchunk], gw3,
                                            axis=mybir.AxisListType.X,
                                            op=OP.add)

                # flush this megablock's ysb columns to keep the tail short
                ytp = yp.tile([128, 128], BF, tag="ytp")
                nc.tensor.transpose(ytp[:nchunk, :],
                                    ysb[:, ycol:ycol + nchunk], ident[:])
                yts = vp.tile([128, 128], F32, tag="yts")
                nc.vector.tensor_copy(yts[:nchunk, :], ytp[:nchunk, :])
                nc.gpsimd.dma_start(out=y_d[ycol:ycol + nchunk, :],
                                    in_=yts[:nchunk, :])
                ycol += nchunk

    nc.compile()
    return nc


def _get_program(has_bias=False):
    if has_bias not in _PROGS:
        _PROGS[has_bias] = _build_program(has_bias)
    return _PROGS[has_bias]


def _host_inputs(x, Wz, bz, Wh, bh, Wl):
    Az = (np.asarray(Wz[0], np.float32) + np.asarray(Wz[1], np.float32))[:F]
    Ah = (np.asarray(Wh[0], np.float32) + np.asarray(Wh[1], np.float32))[:F]
    Acat = np.concatenate([Az, Ah], axis=1)               # [256, 64]
    colscale = np.concatenate([-np.ones(32, np.float32),
                               2 * np.ones(32, np.float32)])
    Acat = Acat * colscale
    acat = np.stack([Acat[:128] * 0.5, Acat[128:] * 0.5]).astype(BF16)
    bsc = np.concatenate([np.asarray(bz, np.float32),
                          np.asarray(bh, np.float32)]) \
        .reshape(2, HID) * colscale.reshape(2, HID)
    # de-interleaved: [bz-scaled x8 | bh-scaled x8]
    biascat8 = np.concatenate([np.tile(bsc[0], 8), np.tile(bsc[1], 8)]) \
        [None, :].astype(BF16)
    wlfull = np.tile(2.0 * np.asarray(Wl, np.float32).reshape(1, HID),
                     (128, 64)).astype(BF16)
    ones = np.ones((1, 128), BF16)
    ident = np.eye(128, dtype=BF16)

    xf = np.asarray(x, np.float32)
    xb0 = np.zeros((NPAD, 128), dtype=F8E3)
    xb0[:N] = (2.0 * xf[:, :128]).astype(F8E3)
    xb1 = np.zeros((NPAD, 128), dtype=F8E3)
    xb1[:N] = (2.0 * xf[:, 128:]).astype(F8E3)

    # per-core shards, then per-megablock transposed contiguous blocks
    def mega_layout(xb):
        sh = xb.reshape(NCORES, PER, 128)
        parts = []
        pos = 0
        for nchunk in BLOCKS:
            nn = nchunk * 128
            blk = sh[:, pos:pos + nn]  # [NCORES, nn, 128]
            parts.append(np.ascontiguousarray(
                blk.transpose(0, 2, 1)).reshape(NCORES, -1))
            pos += nn
        return np.concatenate(parts, axis=1)  # [NCORES, PER*128]

    return (mega_layout(xb0), mega_layout(xb1), acat, biascat8, wlfull,
            ones, ident)


def kernel(x, edge_index, Wz, bz, Wr, br, Wh, bh, Wl, bl):
    from concourse.bass_utils import run_bass_kernel_spmd

    s0, s1, acat, biascat8, wlfull, ones, ident = _host_inputs(
        x, Wz, bz, Wh, bh, Wl)
    has_bias = bool(np.any(np.asarray(bz)) or np.any(np.asarray(bh)))

    nc = _get_program(has_bias)
    in_maps = [{
        "x0": np.ascontiguousarray(s0[i]),
        "x1": np.ascontiguousarray(s1[i]),
        "acat": acat,
        "biascat": biascat8,
        "wlfull": wlfull,
        "ones": ones,
        "ident": ident,
    } for i in range(NCORES)]

    res = run_bass_kernel_spmd(nc, in_maps, core_ids=list(range(NCORES)))

    y = np.concatenate([np.asarray(res.results[i]["y"]).reshape(-1)
                        for i in range(NCORES)])[:N]
    out = (y + np.float32(np.asarray(bl).reshape(-1)[0])).astype(np.float32)
    return out.reshape(N, 1)


# revision 31
# speedup vs baseline: 1.2322x; 1.0219x over previous
"""DCRNN (K=1, H0=0) fused kernel for 8 Trainium2 NeuronCores.

Math (derived from the reference with H0 = 0):
    R is dead (multiplied by H0=0); XH == XHR == [x, 0].
    Az = (Wz[0] + Wz[1])[:F]           # [256, 32]
    Ah = (Wh[0] + Wh[1])[:F]           # [256, 32]
    Zc = sigmoid(-(x @ Az + bz))       # == 1 - Z, strictly positive
    T  = tanh(x @ Ah + bh) = 2*sigmoid(2(x@Ah+bh)) - 1
    h  = relu(Zc * T) == Zc * relu(T)
    y  = h @ Wl + bl                   # [N, 1]

Key tricks vs the old baseline:
  * Mixed precision x: features 0-127 in bf16, features 128-255 in fp8
    e3m4 (scaled by 2 to dodge the subnormal zone; weights pre-divided
    by 2).  HBM traffic drops to 75% (384 B/node).  Measured rel err
    ~1.3e-2 vs the 2e-2 gate.
  * Single activation function: fold -1 into the Az columns and +2 into
    the Ah columns so both gates are plain sigmoid() of the psum -- no
    sigmoid<->tanh ACT table reloads.  tanh path becomes
    (sh - 0.5) * zc * (2*Wl), with the 2 folded into Wl host-side.
  * Biases are zero in this model; the rank-1 bias matmuls are only
    emitted when any bias is nonzero.
  * DMA in 2048-node megablocks (512 KB + 256 KB per block) on the two
    HWDGE queues (sync + scalar) for near-peak HBM bandwidth.
"""

import sys

import numpy as np

sys.path.insert(0, "/opt/trn_rl_repo")

import ml_dtypes

N = 200000
F = 256
HID = 32
NCORES = 8
PER = 25088            # padded nodes per core
NPAD = PER * NCORES    # 200704
MEGA = 4096            # max nodes per megablock (32 chunks of 128)
BLOCKS = [32, 32, 32, 32, 32, 32, 4]
assert sum(BLOCKS) == 196
YCOLS = PER // 128     # 196

BF16 = ml_dtypes.bfloat16
F8E3 = ml_dtypes.float8_e3m4

_PROGS = {}


def _build_program(has_bias=False):
    import concourse.tile as tile
    from concourse import bacc, mybir

    BF = mybir.dt.bfloat16
    F8 = mybir.dt.float8e3
    F32 = mybir.dt.float32
    AF = mybir.ActivationFunctionType
    OP = mybir.AluOpType

    nc = bacc.Bacc("TRN2", target_bir_lowering=False, debug=False,
                   num_devices=NCORES)

    # host feeds per-megablock transposed contiguous blocks; per block the
    # layout is [128 rows, [chunk0 nodes | chunk1 nodes]] in one flat run
    x_d = nc.dram_tensor("x", [PER * 256], F8, kind="ExternalInput").ap()
    acat_d = nc.dram_tensor("acat", [2, 128, 64], BF, kind="ExternalInput").ap()
    bias_d = nc.dram_tensor("biascat", [1, 512], BF, kind="ExternalInput").ap()
    wl_d = nc.dram_tensor("wlfull", [128, 2048], BF, kind="ExternalInput").ap()
    ones_d = nc.dram_tensor("ones", [1, 128], BF, kind="ExternalInput").ap()
    id_d = nc.dram_tensor("ident", [128, 128], BF, kind="ExternalInput").ap()
    y_d = nc.dram_tensor("y", [YCOLS, 128], F32, kind="ExternalOutput").ap()

    with tile.TileContext(nc) as tc:
        with tc.tile_pool(name="const", bufs=1) as cp, \
             tc.tile_pool(name="x0t", bufs=4) as xp0, \
             tc.tile_pool(name="zs", bufs=2) as zp, \
             tc.tile_pool(name="act", bufs=3) as vp, \
             tc.tile_pool(name="ps", bufs=6, space="PSUM") as pp, \
             tc.tile_pool(name="yps", bufs=2, space="PSUM") as yp:

            acat0 = cp.tile([128, 64], BF)
            acat1 = cp.tile([128, 64], BF)
            biascat = cp.tile([1, 512], BF)
            wlfull = cp.tile([128, 2048], BF)
            ones = cp.tile([1, 128], BF)
            ident = cp.tile([128, 128], BF)
            ysb = cp.tile([128, YCOLS], BF)

            nc.scalar.dma_start(out=acat0[:], in_=acat_d[0])
            nc.scalar.dma_start(out=acat1[:], in_=acat_d[1])
            nc.scalar.dma_start(out=biascat[:], in_=bias_d[:])
            nc.scalar.dma_start(out=wlfull[:], in_=wl_d[:])
            nc.scalar.dma_start(out=ones[:], in_=ones_d[:])
            nc.scalar.dma_start(out=ident[:], in_=id_d[:])

            ycol = 0
            for b, nchunk in enumerate(BLOCKS):
                nn = nchunk * 128
                off = ycol * 128 * 128

                xt = xp0.tile([128, 2 * MEGA], F8, tag="xt")
                nc.sync.dma_start(
                    out=xt[:, :2 * nn],
                    in_=x_d[2 * off:2 * off + 256 * nn].rearrange(
                        "(p j) -> p j", p=128))

                # zs layout: [zc for all chunks | sh for all chunks]
                zs = zp.tile([128, MEGA // 2], BF, tag="zs")
                zs2 = zs.rearrange("p (t q) -> p t q", t=2)
                for h in range(0, nchunk, 8):
                    hs = min(8, nchunk - h)
                    ps = pp.tile([128, 512], F32, tag="ps")
                    # de-interleaved psum: cols 0-255 zc-pre, 256-511 sh-pre
                    ps2 = ps.rearrange("p (t q) -> p t q", t=2)
                    if has_bias:
                        nc.tensor.matmul(ps2[:, :, :hs * 32], ones[:],
                                         biascat.rearrange(
                                             "p (t q) -> p t q",
                                             t=2)[:, :, :hs * 32],
                                         start=True, stop=False)
                    for s in range(hs):
                        c = h + s
                        out_sl = ps2[:, :, s * 32:(s + 1) * 32]
                        nc.tensor.matmul(
                            out_sl, xt[:, c * 128:(c + 1) * 128], acat0[:],
                            start=not has_bias, stop=False)
                        nc.tensor.matmul(
                            out_sl, xt[:, nn + c * 128:nn + (c + 1) * 128],
                            acat1[:], start=False, stop=True)

                    nc.scalar.activation(
                        zs2[:, :, h * 32:h * 32 + hs * 32],
                        ps2[:, :, :hs * 32], AF.Sigmoid)

                # batched gating over the whole megablock (dense bf16 ops;
                # relu/mul/reduce run in DVE 4x/2x perf modes)
                t1 = vp.tile([128, MEGA // 4], BF, tag="t1")
                # t1 = (sh - 0.5) * zc ; relu deferred (zc > 0)
                nc.vector.scalar_tensor_tensor(
                    t1[:, :nchunk * 32],
                    zs[:, MEGA // 4:MEGA // 4 + nchunk * 32], 0.5,
                    zs[:, :nchunk * 32],
                    op0=OP.subtract, op1=OP.mult)
                rl = vp.tile([128, MEGA // 4], BF, tag="rl")
                nc.vector.tensor_scalar_max(rl[:, :nchunk * 32],
                                            t1[:, :nchunk * 32], 0.0)
                gw = vp.tile([128, MEGA // 4], BF, tag="gw")
                nc.vector.tensor_mul(gw[:, :nchunk * 32], rl[:, :nchunk * 32],
                                     wlfull[:, :nchunk * 32])
                gw3 = gw[:, :nchunk * 32].rearrange("p (s g) -> p s g", g=32)
                with nc.allow_low_precision(
                        reason="DVE reduce accumulates fp32 internally; "
                               "bf16 output validated vs reference"):
                    nc.vector.tensor_reduce(ysb[:, ycol:ycol + nchunk], gw3,
                                            axis=mybir.AxisListType.X,
                                            op=OP.add)

                # flush this megablock's ysb columns to keep the tail short
                ytp = yp.tile([128, 128], BF, tag="ytp")
                nc.tensor.transpose(ytp[:nchunk, :],
                                    ysb[:, ycol:ycol + nchunk], ident[:])
                yts = vp.tile([128, 128], F32, tag="yts")
                nc.vector.tensor_copy(yts[:nchunk, :], ytp[:nchunk, :])
                nc.gpsimd.dma_start(out=y_d[ycol:ycol + nchunk, :],
                                    in_=yts[:nchunk, :])
                ycol += nchunk

    nc.compile()
    return nc


def _get_program(has_bias=False):
    if has_bias not in _PROGS:
        _PROGS[has_bias] = _build_program(has_bias)
    return _PROGS[has_bias]


def _host_inputs(x, Wz, bz, Wh, bh, Wl):
    Az = (np.asarray(Wz[0], np.float32) + np.asarray(Wz[1], np.float32))[:F]
    Ah = (np.asarray(Wh[0], np.float32) + np.asarray(Wh[1], np.float32))[:F]
    Acat = np.concatenate([Az, Ah], axis=1)               # [256, 64]
    colscale = np.concatenate([-np.ones(32, np.float32),
                               2 * np.ones(32, np.float32)])
    Acat = Acat * colscale
    acat = np.stack([Acat[:128] * 0.5, Acat[128:] * 0.5]).astype(BF16)
    bsc = np.concatenate([np.asarray(bz, np.float32),
                          np.asarray(bh, np.float32)]) \
        .reshape(2, HID) * colscale.reshape(2, HID)
    # de-interleaved: [bz-scaled x8 | bh-scaled x8]
    biascat8 = np.concatenate([np.tile(bsc[0], 8), np.tile(bsc[1], 8)]) \
        [None, :].astype(BF16)
    wlfull = np.tile(2.0 * np.asarray(Wl, np.float32).reshape(1, HID),
                     (128, 64)).astype(BF16)
    ones = np.ones((1, 128), BF16)
    ident = np.eye(128, dtype=BF16)

    xf = np.asarray(x, np.float32)
    xb = np.zeros((NPAD, 256), dtype=F8E3)
    xb[:N] = (2.0 * xf).astype(F8E3)

    # per-core shards, then per-megablock transposed contiguous blocks with
    # per-block layout [128 rows, [chunk0 nodes | chunk1 nodes]]
    sh = xb.reshape(NCORES, PER, 256)
    parts = []
    pos = 0
    for nchunk in BLOCKS:
        nn = nchunk * 128
        blk = sh[:, pos:pos + nn].reshape(NCORES, nn, 2, 128)
        parts.append(np.ascontiguousarray(
            blk.transpose(0, 3, 2, 1)).reshape(NCORES, -1))
        pos += nn
    xflat = np.concatenate(parts, axis=1)  # [NCORES, PER*256]

    return xflat, acat, biascat8, wlfull, ones, ident


def kernel(x, edge_index, Wz, bz, Wr, br, Wh, bh, Wl, bl):
    from concourse.bass_utils import run_bass_kernel_spmd

    s0, acat, biascat8, wlfull, ones, ident = _host_inputs(
        x, Wz, bz, Wh, bh, Wl)
    has_bias = bool(np.any(np.asarray(bz)) or np.any(np.asarray(bh)))

    nc = _get_program(has_bias)
    in_maps = [{
        "x": np.ascontiguousarray(s0[i]),
        "acat": acat,
        "biascat": biascat8,
        "wlfull": wlfull,
        "ones": ones,
        "ident": ident,
    } for i in range(NCORES)]

    res = run_bass_kernel_spmd(nc, in_maps, core_ids=list(range(NCORES)))

    y = np.concatenate([np.asarray(res.results[i]["y"]).reshape(-1)
                        for i in range(NCORES)])[:N]
    out = (y + np.float32(np.asarray(bl).reshape(-1)[0])).astype(np.float32)
    return out.reshape(N, 1)


# revision 33
# speedup vs baseline: 1.2877x; 1.0451x over previous
"""DCRNN (K=1, H0=0) fused kernel for 8 Trainium2 NeuronCores.

Math (derived from the reference with H0 = 0):
    R is dead (multiplied by H0=0); XH == XHR == [x, 0].
    Az = (Wz[0] + Wz[1])[:F]           # [256, 32]
    Ah = (Wh[0] + Wh[1])[:F]           # [256, 32]
    Zc = sigmoid(-(x @ Az + bz))       # == 1 - Z, strictly positive
    T  = tanh(x @ Ah + bh) = 2*sigmoid(2(x@Ah+bh)) - 1
    h  = relu(Zc * T) == Zc * relu(T)
    y  = h @ Wl + bl                   # [N, 1]

Key tricks vs the old baseline:
  * Mixed precision x: features 0-127 in bf16, features 128-255 in fp8
    e3m4 (scaled by 2 to dodge the subnormal zone; weights pre-divided
    by 2).  HBM traffic drops to 75% (384 B/node).  Measured rel err
    ~1.3e-2 vs the 2e-2 gate.
  * Single activation function: fold -1 into the Az columns and +2 into
    the Ah columns so both gates are plain sigmoid() of the psum -- no
    sigmoid<->tanh ACT table reloads.  tanh path becomes
    (sh - 0.5) * zc * (2*Wl), with the 2 folded into Wl host-side.
  * Biases are zero in this model; the rank-1 bias matmuls are only
    emitted when any bias is nonzero.
  * DMA in 2048-node megablocks (512 KB + 256 KB per block) on the two
    HWDGE queues (sync + scalar) for near-peak HBM bandwidth.
"""

import sys

import numpy as np

sys.path.insert(0, "/opt/trn_rl_repo")

import ml_dtypes

N = 200000
F = 256
HID = 32
NCORES = 8
PER = 25088            # padded nodes per core
NPAD = PER * NCORES    # 200704
MEGA = 4096            # max nodes per megablock (32 chunks of 128)
BLOCKS = [8, 24, 32, 32, 32, 32, 32, 4]
assert sum(BLOCKS) == 196
YCOLS = PER // 128     # 196

BF16 = ml_dtypes.bfloat16
F8E3 = ml_dtypes.float8_e3m4

_PROGS = {}


def _build_program(has_bias=False):
    import concourse.tile as tile
    from concourse import bacc, mybir

    BF = mybir.dt.bfloat16
    F8 = mybir.dt.float8e3
    F32 = mybir.dt.float32
    AF = mybir.ActivationFunctionType
    OP = mybir.AluOpType

    nc = bacc.Bacc("TRN2", target_bir_lowering=False, debug=False,
                   num_devices=NCORES)

    # host feeds per-megablock transposed contiguous blocks; per block the
    # layout is [128 rows, [chunk0 nodes | chunk1 nodes]] in one flat run
    x_d = nc.dram_tensor("x", [PER * 256], F8, kind="ExternalInput").ap()
    acat_d = nc.dram_tensor("acat", [2, 128, 64], BF, kind="ExternalInput").ap()
    bias_d = nc.dram_tensor("biascat", [1, 512], BF, kind="ExternalInput").ap()
    wl_d = nc.dram_tensor("wlfull", [128, 2048], BF, kind="ExternalInput").ap()
    ones_d = nc.dram_tensor("ones", [1, 128], BF, kind="ExternalInput").ap()
    y_d = nc.dram_tensor("y", [128, YCOLS], BF, kind="ExternalOutput").ap()

    with tile.TileContext(nc) as tc:
        with tc.tile_pool(name="const", bufs=1) as cp, \
             tc.tile_pool(name="x0t", bufs=4) as xp0, \
             tc.tile_pool(name="zs", bufs=2) as zp, \
             tc.tile_pool(name="act", bufs=3) as vp, \
             tc.tile_pool(name="ps", bufs=6, space="PSUM") as pp:

            acat0 = cp.tile([128, 64], BF)
            acat1 = cp.tile([128, 64], BF)
            biascat = cp.tile([1, 512], BF)
            wlfull = cp.tile([128, 2048], BF)
            ones = cp.tile([1, 128], BF)
            ysb = cp.tile([128, YCOLS], BF)

            nc.scalar.dma_start(out=acat0[:], in_=acat_d[0])
            nc.scalar.dma_start(out=acat1[:], in_=acat_d[1])
            nc.scalar.dma_start(out=wlfull[:], in_=wl_d[:])
            if has_bias:
                nc.scalar.dma_start(out=biascat[:], in_=bias_d[:])
                nc.scalar.dma_start(out=ones[:], in_=ones_d[:])

            ycol = 0
            for b, nchunk in enumerate(BLOCKS):
                nn = nchunk * 128
                off = ycol * 128 * 128

                xt = xp0.tile([128, 2 * MEGA], F8, tag="xt")
                nc.sync.dma_start(
                    out=xt[:, :2 * nn],
                    in_=x_d[2 * off:2 * off + 256 * nn].rearrange(
                        "(p j) -> p j", p=128))

                # zs layout: [zc for all chunks | sh for all chunks]
                zs = zp.tile([128, MEGA // 2], BF, tag="zs")
                zs2 = zs.rearrange("p (t q) -> p t q", t=2)
                for h in range(0, nchunk, 8):
                    hs = min(8, nchunk - h)
                    ps = pp.tile([128, 512], F32, tag="ps")
                    # de-interleaved psum: cols 0-255 zc-pre, 256-511 sh-pre
                    ps2 = ps.rearrange("p (t q) -> p t q", t=2)
                    if has_bias:
                        nc.tensor.matmul(ps2[:, :, :hs * 32], ones[:],
                                         biascat.rearrange(
                                             "p (t q) -> p t q",
                                             t=2)[:, :, :hs * 32],
                                         start=True, stop=False)
                    for s in range(hs):
                        c = h + s
                        out_sl = ps2[:, :, s * 32:(s + 1) * 32]
                        nc.tensor.matmul(
                            out_sl, xt[:, c * 128:(c + 1) * 128], acat0[:],
                            start=not has_bias, stop=False)
                        nc.tensor.matmul(
                            out_sl, xt[:, nn + c * 128:nn + (c + 1) * 128],
                            acat1[:], start=False, stop=True)

                    nc.scalar.activation(
                        zs2[:, :, h * 32:h * 32 + hs * 32],
                        ps2[:, :, :hs * 32], AF.Sigmoid)

                # batched gating over the whole megablock (dense bf16 ops;
                # relu/mul/reduce run in DVE 4x/2x perf modes)
                t1 = vp.tile([128, MEGA // 4], BF, tag="t1")
                # t1 = (sh - 0.5) * zc ; relu deferred (zc > 0)
                nc.vector.scalar_tensor_tensor(
                    t1[:, :nchunk * 32],
                    zs[:, MEGA // 4:MEGA // 4 + nchunk * 32], 0.5,
                    zs[:, :nchunk * 32],
                    op0=OP.subtract, op1=OP.mult)
                rl = vp.tile([128, MEGA // 4], BF, tag="rl")
                nc.vector.tensor_scalar_max(rl[:, :nchunk * 32],
                                            t1[:, :nchunk * 32], 0.0)
                gw = vp.tile([128, MEGA // 4], BF, tag="gw")
                nc.vector.tensor_mul(gw[:, :nchunk * 32], rl[:, :nchunk * 32],
                                     wlfull[:, :nchunk * 32])
                gw3 = gw[:, :nchunk * 32].rearrange("p (s g) -> p s g", g=32)
                with nc.allow_low_precision(
                        reason="DVE reduce accumulates fp32 internally; "
                               "bf16 output validated vs reference"):
                    nc.vector.tensor_reduce(ysb[:, ycol:ycol + nchunk], gw3,
                                            axis=mybir.AxisListType.X,
                                            op=OP.add)

                ycol += nchunk

            # single untransposed y store; host transposes
            nc.gpsimd.dma_start(out=y_d[:, :], in_=ysb[:, :])

    nc.compile()
    return nc


def _get_program(has_bias=False):
    if has_bias not in _PROGS:
        _PROGS[has_bias] = _build_program(has_bias)
    return _PROGS[has_bias]


def _host_inputs(x, Wz, bz, Wh, bh, Wl):
    Az = (np.asarray(Wz[0], np.float32) + np.asarray(Wz[1], np.float32))[:F]
    Ah = (np.asarray(Wh[0], np.float32) + np.asarray(Wh[1], np.float32))[:F]
    Acat = np.concatenate([Az, Ah], axis=1)               # [256, 64]
    colscale = np.concatenate([-np.ones(32, np.float32),
                               2 * np.ones(32, np.float32)])
    Acat = Acat * colscale
    acat = np.stack([Acat[:128] * 0.5, Acat[128:] * 0.5]).astype(BF16)
    bsc = np.concatenate([np.asarray(bz, np.float32),
                          np.asarray(bh, np.float32)]) \
        .reshape(2, HID) * colscale.reshape(2, HID)
    # de-interleaved: [bz-scaled x8 | bh-scaled x8]
    biascat8 = np.concatenate([np.tile(bsc[0], 8), np.tile(bsc[1], 8)]) \
        [None, :].astype(BF16)
    wlfull = np.tile(2.0 * np.asarray(Wl, np.float32).reshape(1, HID),
                     (128, 64)).astype(BF16)
    ones = np.ones((1, 128), BF16)

    xf = np.asarray(x, np.float32)
    xb = np.zeros((NPAD, 256), dtype=F8E3)
    xb[:N] = (2.0 * xf).astype(F8E3)

    # per-core shards, then per-megablock transposed contiguous blocks with
    # per-block layout [128 rows, [chunk0 nodes | chunk1 nodes]]
    sh = xb.reshape(NCORES, PER, 256)
    parts = []
    pos = 0
    for nchunk in BLOCKS:
        nn = nchunk * 128
        blk = sh[:, pos:pos + nn].reshape(NCORES, nn, 2, 128)
        parts.append(np.ascontiguousarray(
            blk.transpose(0, 3, 2, 1)).reshape(NCORES, -1))
        pos += nn
    xflat = np.concatenate(parts, axis=1)  # [NCORES, PER*256]

    return xflat, acat, biascat8, wlfull, ones


def kernel(x, edge_index, Wz, bz, Wr, br, Wh, bh, Wl, bl):
    from concourse.bass_utils import run_bass_kernel_spmd

    s0, acat, biascat8, wlfull, ones = _host_inputs(
        x, Wz, bz, Wh, bh, Wl)
    has_bias = bool(np.any(np.asarray(bz)) or np.any(np.asarray(bh)))

    nc = _get_program(has_bias)
    in_maps = [{
        "x": np.ascontiguousarray(s0[i]),
        "acat": acat,
        "biascat": biascat8,
        "wlfull": wlfull,
        "ones": ones,
    } for i in range(NCORES)]

    res = run_bass_kernel_spmd(nc, in_maps, core_ids=list(range(NCORES)))

    y = np.concatenate([np.asarray(res.results[i]["y"])
                        .astype(np.float32).T.reshape(-1)
                        for i in range(NCORES)])[:N]
    out = (y + np.float32(np.asarray(bl).reshape(-1)[0])).astype(np.float32)
    return out.reshape(N, 1)
